# revision 23
# baseline (speedup 1.0000x reference)
"""MDTA Trainium2 kernel: 8 cores = 4 samples x 2 head-groups.

v1: unfolded qkv (1x1 + banded depthwise-3x3 fused in SBUF), norms fused
into producer stages, gram via DMA-transpose + SBUF-resident q^T/k^T,
bf16 inputs/outputs, per-iteration memsets eliminated, PSUM-evacuation
spread across Activation/DVE/Pool engines.
"""
import numpy as np
import ml_dtypes
import json as _json
import concourse.bass as bass

# Patch Bass.to_json_bytes: split multi-sem-waits onto same-engine NoOps
# (this walrus build rejects instructions with >1 sync wait).
_orig_tjb = bass.Bass.to_json_bytes
_wctr = [0]

def _split_waits(block):
    out = []
    for ins in block.get("instructions", []):
        si = ins.get("sync_info")
        waits = (si or {}).get("on_wait") or []
        if len(waits) > 1:
            si["on_wait"] = waits[-1:]
            for w in waits[:-1]:
                _wctr[0] += 1
                out.append({"debug": ins.get("debug", 0), "engine": ins["engine"],
                            "ins": [], "outs": [], "name": f"wsplit-{_wctr[0]}",
                            "opcode": "NoOp",
                            "sync_info": {"on_update": [], "on_wait": [w]}})
        out.append(ins)
    block["instructions"] = out
    for sub in block.get("blocks", []):
        _split_waits(sub)

def _patched_tjb(self):
    d = _json.loads(_orig_tjb(self))
    for fn in d.get("functions", []):
        for b in fn.get("blocks", []):
            _split_waits(b)
    return _json.dumps(d).encode()

if not getattr(bass.Bass, "_waitfix_done", False):
    bass.Bass.to_json_bytes = _patched_tjb
    bass.Bass._waitfix_done = True
import concourse.mybir as mybir
from concourse.tile import TileContext
from concourse.bass_utils import run_bass_kernel_spmd

BF = mybir.dt.bfloat16
F32 = mybir.dt.float32
H, W, C = 192, 192, 256
HW = H * W
S = 98  # subband size

DEC_LO = np.array([0.035226291882100656, -0.085441273882241486, -0.13501102001039084,
                   0.45987750211933132, 0.80689150931333875, 0.33267055295095688], dtype=np.float64)
DEC_HI = np.array([-0.33267055295095688, 0.80689150931333875, -0.45987750211933132,
                   -0.13501102001039084, 0.085441273882241486, 0.035226291882100656], dtype=np.float64)
H0A = DEC_LO[::-1].copy()
H1A = DEC_HI[::-1].copy()
G0S = DEC_LO.copy()  # REC_LO reversed = DEC_LO
G1S = np.array([0.035226291882100656, 0.085441273882241486, -0.13501102001039084,
                -0.45987750211933132, 0.80689150931333875, -0.33267055295095688], dtype=np.float64)[::-1].copy()


STAGE_MARKS = []


def _mark(nc, stage):
    STAGE_MARKS.append((int(nc.get_next_instruction_name().split("-")[1]), stage))


_rot = [0]


def _copy(nc, out, in_):
    # PSUM evacuation: GPSIMD/Pool cannot access PSUM, so rotate Act/DVE only.
    r = _rot[0] % 2
    _rot[0] += 1
    if r == 0:
        nc.scalar.copy(out, in_)
    else:
        nc.vector.tensor_copy(out, in_)


def build_core_kernel():
    nc = bass.Bass("TRN2")
    STAGE_MARKS.clear()
    _rot[0] = 0
    # inputs (per core)
    xk0 = nc.dram_tensor("xk0", [128, H, W], BF, kind="ExternalInput")
    xk1 = nc.dram_tensor("xk1", [128, H, W], BF, kind="ExternalInput")
    xq = nc.dram_tensor("xq", [128, H, W], BF, kind="ExternalInput")  # local 128 ch
    w1x1 = nc.dram_tensor("w1x1", [2, 128, 256], BF, kind="ExternalInput")  # [in-half, in, out(k|v)]
    dwk = nc.dram_tensor("dwk", [9, 128, 128], BF, kind="ExternalInput")  # diag dw taps, k half
    dwv = nc.dram_tensor("dwv", [9, 128, 128], BF, kind="ExternalInput")  # diag dw taps, v half
    taps_ab = nc.dram_tensor("taps_ab", [12, 128, 128], BF, kind="ExternalInput")
    taps_de = nc.dram_tensor("taps_de", [12, 128, 128], BF, kind="ExternalInput")
    dwq = nc.dram_tensor("dwq", [4, 9, 128, 128], BF, kind="ExternalInput")
    projlt = nc.dram_tensor("projlt", [128, 256], BF, kind="ExternalInput")
    tempv = nc.dram_tensor("tempv", [128, 1], F32, kind="ExternalInput")
    identb = nc.dram_tensor("identb", [128, 128], BF, kind="ExternalInput")
    y = nc.dram_tensor("y", [2, 128, HW], BF, kind="ExternalOutput")
    # DRAM scratch
    vd = nc.dram_tensor("vd", [128, HW], BF)
    qd = nc.dram_tensor("qd", [128, H, W], BF)
    loh = nc.dram_tensor("loh", [128, 2, H, S], BF)
    subb = nc.dram_tensor("subb", [128, 4, S, 100], BF)  # W-padded (cols 0,99 zeroed once)
    zq = nc.dram_tensor("zq", [128, 4, S, S], BF)
    synth = nc.dram_tensor("synth", [128, 2, 2, 96, S], BF)

    with TileContext(nc) as tc:
        with tc.tile_pool(name="const", bufs=1) as cpool:
            # ---- constants
            t_ab = cpool.tile([128, 12, 128], BF)
            nc.sync.dma_start(out=t_ab[:, :, :], in_=taps_ab.rearrange("t p c -> p t c"))
            t_de = cpool.tile([128, 12, 128], BF)
            nc.sync.dma_start(out=t_de[:, :, :], in_=taps_de.rearrange("t p c -> p t c"))
            t_dw = cpool.tile([128, 36, 128], BF)
            nc.sync.dma_start(out=t_dw[:, :, :], in_=dwq.rearrange("s t p c -> p (s t) c"))
            t_w1 = cpool.tile([128, 2, 256], BF)
            nc.sync.dma_start(out=t_w1[:, :, :], in_=w1x1.rearrange("h p c -> p h c"))
            t_dwk = cpool.tile([128, 9, 128], BF)
            nc.sync.dma_start(out=t_dwk[:, :, :], in_=dwk.rearrange("t p c -> p t c"))
            t_dwv = cpool.tile([128, 9, 128], BF)
            nc.sync.dma_start(out=t_dwv[:, :, :], in_=dwv.rearrange("t p c -> p t c"))
            t_proj = cpool.tile([128, 256], BF)
            nc.sync.dma_start(out=t_proj[:, :], in_=projlt[:, :])
            t_id = cpool.tile([128, 128], BF)
            nc.sync.dma_start(out=t_id[:, :], in_=identb[:, :])
            t_temp = cpool.tile([128, 1], F32)
            nc.sync.dma_start(out=t_temp[:, :], in_=tempv[:, :])

            knp = cpool.tile([128, 16], F32)
            qnp = cpool.tile([128, 20], F32)
            nc.vector.memset(knp[:, :], 0)
            nc.vector.memset(qnp[:, :], 0)
            mt_ = cpool.tile([128, 256], BF)     # attention+proj matrix (gram phase -> y phase)
            kdT = cpool.tile([128, 288, 128], BF)  # transposed k, SBUF-resident

            _mark(nc, "kv")
            # ======== kv: 1x1 (C=256 -> k|v 128+128) + depthwise 3x3, fused per 12-row band
            BKV, NB = 12, 16
            with tc.tile_pool(name="kvw", bufs=2) as kp, \
                 tc.tile_pool(name="kvps", bufs=1, space="PSUM") as pp1:
                for b in range(NB):
                    r0 = BKV * b
                    xb0 = kp.tile([128, 14, 192], BF, tag="xb0")
                    xb1 = kp.tile([128, 14, 192], BF, tag="xb1")
                    v0, v1 = max(0, r0 - 1), min(H, r0 + BKV + 1)
                    if b == 0:
                        nc.vector.memset(xb0[:, 0, :], 0)
                        nc.vector.memset(xb1[:, 0, :], 0)
                    if b == NB - 1:
                        nc.vector.memset(xb0[:, 13, :], 0)
                        nc.vector.memset(xb1[:, 13, :], 0)
                    nc.sync.dma_start(out=xb0[:, v0 - (r0 - 1):v1 - (r0 - 1), :], in_=xk0[:, v0:v1, :])
                    nc.sync.dma_start(out=xb1[:, v0 - (r0 - 1):v1 - (r0 - 1), :], in_=xk1[:, v0:v1, :])
                    kvp = [kp.tile([128, 14, 194], BF, tag="kvp0", name="kvp0"),
                           kp.tile([128, 14, 194], BF, tag="kvp1", name="kvp1")]
                    for mt in range(2):  # zero W-pad columns (tiny, every band)
                        nc.vector.memset(kvp[mt][:, :, 0:1], 0)
                        nc.vector.memset(kvp[mt][:, :, 193:194], 0)
                    for mt in range(2):
                        for i in range(7):
                            ps = pp1.tile([128, 2, 192], F32, tag="ps1", bufs=2, name="ps1")
                            nc.tensor.matmul(ps[:, :, :], t_w1[:, 0, 128 * mt:128 * mt + 128],
                                             xb0[:, 2 * i:2 * i + 2, :], start=True, stop=False)
                            nc.tensor.matmul(ps[:, :, :], t_w1[:, 1, 128 * mt:128 * mt + 128],
                                             xb1[:, 2 * i:2 * i + 2, :], start=False, stop=True)
                            _copy(nc, kvp[mt][:, 2 * i:2 * i + 2, 1:193], ps[:, :, :])
                    for mt in range(2):
                        wt = t_dwk if mt == 0 else t_dwv
                        psd = [pp1.tile([128, 2, 192], F32, tag="psdw", bufs=6, name="psd")
                               for _ in range(6)]
                        for t9 in range(9):
                            u, v = divmod(t9, 3)
                            for j in range(6):
                                nc.tensor.matmul(psd[j][:, :, :], wt[:, t9, :],
                                                 kvp[mt][:, 2 * j + u:2 * j + u + 2, v:v + 192],
                                                 start=(t9 == 0), stop=(t9 == 8))
                        out = kp.tile([128, 12, 192], BF, tag=f"okv{mt}", name="okv")
                        for j in range(6):
                            _copy(nc, out[:, 2 * j:2 * j + 2, :], psd[j][:, :, :])
                        if mt == 0:
                            sqk = kp.tile([128, 12, 192], BF, tag="sqk")
                            nc.vector.scalar_tensor_tensor(sqk[:, :, :], out[:, :, :], 1.0,
                                                           out[:, :, :], mybir.AluOpType.mult,
                                                           mybir.AluOpType.mult,
                                                           accum_out=knp[:, b:b + 1])
                            nc.sync.dma_start_transpose(
                                out=kdT[:, 18 * b:18 * b + 18, :],
                                in_=out.rearrange("p r w -> p (r w)"))
                        else:
                            nc.sync.dma_start(out=vd[:, r0 * W:(r0 + BKV) * W],
                                              in_=out.rearrange("p r w -> p (r w)"))
                    if b == 0:  # zero subb (pad cols) via Act queue, overlapped with kv
                        zsrc = kp.tile([128, 1960], BF, tag="zsrc", bufs=1)
                        nc.vector.memset(zsrc[:, :], 0)
                        for sb in range(4):
                            for j in range(5):
                                nc.scalar.dma_start(
                                    out=subb[:, sb, :, :].rearrange("p r w -> p (r w)")[:, 1960 * j:1960 * j + 1960],
                                    in_=zsrc[:, :])

            # ======== wavelet query path (c-parts diag matmuls)
            with tc.tile_pool(name="ww", bufs=2) as pool, \
                 tc.tile_pool(name="wps", bufs=7, space="PSUM") as pp:
                _mark(nc, "stageA")
                # stage A: W-analysis (x -> lo/hi)
                RA = 5
                for it, r0 in enumerate(range(0, H, RA)):
                    rr = min(RA, H - r0)
                    xt = pool.tile([128, RA, 202], BF, tag="xa", bufs=4)
                    nc.vector.memset(xt[:, :rr, 0:4], 0)
                    nc.vector.memset(xt[:, :rr, 196:202], 0)
                    nc.sync.dma_start(out=xt[:, :rr, 4:196], in_=xq[:, r0:r0 + rr, :])
                    for f in range(2):
                        ps = pp.tile([128, RA, S], F32, tag="ps")
                        for t in range(6):
                            rhs = xt[:, :rr, t:t + 196].rearrange("p r (j two) -> p two r j", two=2)[:, 0]
                            nc.tensor.matmul(ps[:, :rr, :], t_ab[:, 6 * f + t, :], rhs,
                                             start=(t == 0), stop=(t == 5))
                        ot = pool.tile([128, RA, S], BF, tag="oA", bufs=3)
                        _copy(nc, ot[:, :rr, :], ps[:, :rr, :])
                        nc.sync.dma_start(out=loh[:, f, r0:r0 + rr, :], in_=ot[:, :rr, :])

                _mark(nc, "stageB")
                # stage B: H-analysis (lo/hi -> 4 subbands)
                RB = 5
                for m0 in range(0, S, RB):
                    mm = min(RB, S - m0)
                    lo_r0 = 2 * m0 - 4
                    nrows = 2 * mm + 5
                    bt = pool.tile([128, 2, 2 * RB + 5, S], BF, tag="xb", bufs=4)
                    v0 = max(0, lo_r0)
                    v1 = min(H, lo_r0 + nrows)
                    if v0 > lo_r0 or v1 < lo_r0 + nrows:
                        nc.vector.memset(bt[:, :, :, :], 0)
                    nc.sync.dma_start(out=bt[:, :, v0 - lo_r0:v1 - lo_r0, :],
                                      in_=loh[:, :, v0:v1, :])
                    for sb in range(4):
                        f_h, src = (sb % 2), (sb // 2)
                        ps = pp.tile([128, RB, S], F32, tag="ps")
                        for t in range(6):
                            rhs = bt[:, src, t:t + 2 * mm, :].rearrange("p (m two) j -> p two m j", two=2)[:, 0]
                            nc.tensor.matmul(ps[:, :mm, :], t_ab[:, 6 * f_h + t, :], rhs,
                                             start=(t == 0), stop=(t == 5))
                        ot = pool.tile([128, RB, S], BF, tag="oB", bufs=3)
                        _copy(nc, ot[:, :mm, :], ps[:, :mm, :])
                        nc.sync.dma_start(out=subb[:, sb, m0:m0 + mm, 1:99], in_=ot[:, :mm, :])

                _mark(nc, "stageC")
                # stage C: depthwise 3x3, all 4 subbands per row-chunk (one shared load)
                RC_ = 5
                for it, m0 in enumerate(range(0, S, RC_)):
                    mm = min(RC_, S - m0)
                    ct = pool.tile([128, 4, RC_ + 2, 100], BF, tag="xc", bufs=4)
                    v0 = max(0, m0 - 1)
                    v1 = min(S, m0 + mm + 1)
                    if v0 > m0 - 1 or v1 < m0 + mm + 1:
                        nc.vector.memset(ct[:, :, :, :], 0)
                    nc.sync.dma_start(out=ct[:, :, v0 - (m0 - 1):v1 - (m0 - 1), :],
                                      in_=subb[:, :, v0:v1, :])
                    for sb in range(4):
                        ps = pp.tile([128, RC_, S], F32, tag="ps")
                        for u in range(3):
                            for v in range(3):
                                t = 3 * u + v
                                nc.tensor.matmul(ps[:, :mm, :], t_dw[:, 9 * sb + t, :],
                                                 ct[:, sb, u:u + mm, v:v + 98],
                                                 start=(t == 0), stop=(t == 8))
                        ot = pool.tile([128, RC_, S], BF, tag="oC", bufs=3)
                        _copy(nc, ot[:, :mm, :], ps[:, :mm, :])
                        nc.sync.dma_start(out=zq[:, sb, m0:m0 + mm, :], in_=ot[:, :mm, :])

                _mark(nc, "stageD")
                # stage D: H-synthesis -> lo2/hi2 parity planes (one zq load per row-chunk)
                RD = 5
                for a0 in range(0, 96, RD):
                    aa = min(RD, 96 - a0)
                    dt_ = pool.tile([128, 4, RD + 2, S], BF, tag="xd", bufs=4)
                    v1 = min(S, a0 + aa + 2)
                    nc.sync.dma_start(out=dt_[:, :, :v1 - a0, :],
                                      in_=zq[:, :, a0:v1, :])
                    for fo in range(2):
                        for pr in range(2):
                            ps = pp.tile([128, RD, S], F32, tag="ps")
                            for src in range(2):
                                for d in range(3):
                                    ti = 6 * src + (2 * d + 1 - pr)
                                    nc.tensor.matmul(ps[:, :aa, :], t_de[:, ti, :],
                                                     dt_[:, 2 * fo + src, d:d + aa, :],
                                                     start=(src == 0 and d == 0),
                                                     stop=(src == 1 and d == 2))
                            ot = pool.tile([128, RD, S], BF, tag="oD", bufs=3)
                            _copy(nc, ot[:, :aa, :], ps[:, :aa, :])
                            nc.sync.dma_start(out=synth[:, fo, pr, a0:a0 + aa, :], in_=ot[:, :aa, :])

                _mark(nc, "stageE")
                # stage E: W-synthesis -> q rows (natural order) + qnorm + qd
                RE = 5
                for a0 in range(0, 96, RE):
                    aa = min(RE, 96 - a0)
                    qsb = pool.tile([128, 2 * RE, W], BF, tag="qE")
                    for pr in range(2):
                        et = pool.tile([128, 2, RE, S], BF, tag="xe", bufs=4)
                        nc.sync.dma_start(out=et[:, :, :aa, :], in_=synth[:, :, pr, a0:a0 + aa, :])
                        for pc in range(2):
                            ps = pp.tile([128, RE, 96], F32, tag="ps")
                            for src in range(2):
                                for d in range(3):
                                    ti = 6 * src + (2 * d + 1 - pc)
                                    nc.tensor.matmul(ps[:, :aa, :], t_de[:, ti, :],
                                                     et[:, src, :aa, d:d + 96],
                                                     start=(src == 0 and d == 0),
                                                     stop=(src == 1 and d == 2))
                            dst = qsb.rearrange("p (r two) w -> p two r w", two=2)[:, pr, :aa]
                            dst2 = dst.rearrange("p r (j two) -> p two r j", two=2)[:, pc]
                            _copy(nc, dst2, ps[:, :aa, :])
                    sqq = pool.tile([128, 2 * RE, W], BF, tag="sqq")
                    nc.vector.scalar_tensor_tensor(sqq[:, :2 * aa, :], qsb[:, :2 * aa, :], 1.0,
                                                   qsb[:, :2 * aa, :], mybir.AluOpType.mult,
                                                   mybir.AluOpType.mult,
                                                   accum_out=qnp[:, a0 // RE:a0 // RE + 1])
                    nc.sync.dma_start(out=qd[:, 2 * a0:2 * a0 + 2 * aa, :], in_=qsb[:, :2 * aa, :])

            _mark(nc, "gram")
            # ======== gram + attention block
            with tc.tile_pool(name="gw", bufs=1) as gw, \
                 tc.tile_pool(name="gps", bufs=1, space="PSUM") as gp:
                qdT = gw.tile([128, 288, 128], BF)
                g_ps = gp.tile([128, 128], F32, tag="g")
                qd_flat = qd.rearrange("p h w -> p (h w)")
                for c in range(4):
                    nc.sync.dma_start_transpose(out=qdT[:, 72 * c:72 * c + 72, :],
                                                in_=qd_flat[:, 9216 * c:9216 * c + 9216])
                    for i in range(72 * c, 72 * c + 72):
                        nc.tensor.matmul(g_ps[:, :], qdT[:, i, :], kdT[:, i, :],
                                         start=(i == 0), stop=(i == 287))

                _mark(nc, "attn")
                qn = gw.tile([128, 1], F32)
                kn = gw.tile([128, 1], F32)
                nc.vector.tensor_reduce(qn[:, :], qnp[:, :], axis=mybir.AxisListType.X,
                                        op=mybir.AluOpType.add)
                nc.vector.tensor_reduce(kn[:, :], knp[:, :], axis=mybir.AxisListType.X,
                                        op=mybir.AluOpType.add)
                nc.scalar.sqrt(qn[:, :], qn[:, :])
                nc.scalar.sqrt(kn[:, :], kn[:, :])
                nc.vector.tensor_scalar_max(qn[:, :], qn[:, :], 1e-12)
                nc.vector.tensor_scalar_max(kn[:, :], kn[:, :], 1e-12)
                rq = gw.tile([128, 1], F32)
                rk = gw.tile([128, 1], F32)
                nc.vector.reciprocal(rq[:, :], qn[:, :])
                nc.vector.reciprocal(rk[:, :], kn[:, :])
                nc.vector.tensor_mul(rq[:, :], rq[:, :], t_temp[:, :])

                gsb = gw.tile([128, 128], BF)
                nc.vector.tensor_scalar_mul(gsb[:, :], g_ps[:, :], rq[:, :])
                pt = gp.tile([128, 128], BF, tag="pt", bufs=2)
                nc.tensor.transpose(pt[:, :], gsb[:, :], t_id[:, :])
                gtb = gw.tile([128, 128], BF)
                nc.scalar.activation(gtb[:, :], pt[:, :], mybir.ActivationFunctionType.Copy,
                                     scale=rk[:, :])
                pt2 = gp.tile([128, 128], BF, tag="pt", bufs=2)
                nc.tensor.transpose(pt2[:, :], gtb[:, :], t_id[:, :])

                eb = gw.tile([128, 32], F32)
                for h in range(4):
                    nc.scalar.activation(eb[32 * h:32 * h + 32, :],
                                         pt2[32 * h:32 * h + 32, 32 * h:32 * h + 32],
                                         mybir.ActivationFunctionType.Exp)
                ssum = gw.tile([128, 1], F32)
                nc.vector.tensor_reduce(ssum[:, :], eb[:, :], axis=mybir.AxisListType.X,
                                        op=mybir.AluOpType.add)
                rs = gw.tile([128, 1], F32)
                nc.vector.reciprocal(rs[:, :], ssum[:, :])
                nc.vector.tensor_scalar_mul(eb[:, :], eb[:, :], rs[:, :])
                bd = gw.tile([128, 128], BF)
                nc.vector.memset(bd[:, :], 0)
                for h in range(4):
                    nc.vector.tensor_copy(bd[32 * h:32 * h + 32, 32 * h:32 * h + 32],
                                          eb[32 * h:32 * h + 32, :])

                mps = gp.tile([128, 256], F32, tag="mps")
                nc.tensor.matmul(mps[:, :], bd[:, :], t_proj[:, :], start=True, stop=True)
                nc.scalar.copy(mt_[:, :], mps[:, :])

            _mark(nc, "y")
            # ======== y = M @ v (2KB-per-partition DMA granularity)
            with tc.tile_pool(name="yw", bufs=2) as yp, \
                 tc.tile_pool(name="yps", bufs=8, space="PSUM") as yq:
                for i in range(18):
                    vt = yp.tile([128, 2048], BF, tag="vt", bufs=6)
                    nc.sync.dma_start(out=vt[:, :], in_=vd[:, 2048 * i:2048 * i + 2048])
                    yst = [yp.tile([128, 2048], BF, tag="yst0", name="yst0", bufs=3),
                           yp.tile([128, 2048], BF, tag="yst1", name="yst1", bufs=3)]
                    for j in range(4):
                        for mtile in range(2):
                            ps = yq.tile([128, 512], F32, tag="ps")
                            nc.tensor.matmul(ps[:, :], mt_[:, 128 * mtile:128 * mtile + 128],
                                             vt[:, 512 * j:512 * j + 512], start=True, stop=True)
                            _copy(nc, yst[mtile][:, 512 * j:512 * j + 512], ps[:, :])
                    for mtile in range(2):
                        nc.sync.dma_start(out=y[mtile, :, 2048 * i:2048 * i + 2048],
                                          in_=yst[mtile][:, :])
    return nc


def _prep_core(x, qkv_w, qkv_conv_w, conv5_w, conv7_w, conv9_w, proj_w, temperature, b, g):
    bf = ml_dtypes.bfloat16
    xb = np.asarray(x[b], np.float32)
    sl = slice(128 * g, 128 * g + 128)
    qkv_loc = np.concatenate([qkv_w[sl], qkv_w[256 + 128 * g:256 + 128 * g + 128]], 0)  # (256 out, 256 in)
    conv_loc = np.concatenate([qkv_conv_w[sl, 0], qkv_conv_w[256 + 128 * g:256 + 128 * g + 128, 0]], 0)  # (256,3,3)
    w1 = qkv_loc.T.reshape(2, 128, 256).copy()  # [in-half, in(128), out(256)]
    dwk = np.zeros((9, 128, 128), np.float32)
    dwv = np.zeros((9, 128, 128), np.float32)
    for t in range(9):
        u, v = divmod(t, 3)
        dwk[t] = np.diag(conv_loc[:128, u, v])
        dwv[t] = np.diag(conv_loc[128:, u, v])
    taps_ab = np.zeros((12, 128, 128), np.float32)
    taps_de = np.zeros((12, 128, 128), np.float32)
    eye = np.eye(128, dtype=np.float32)
    for t in range(6):
        taps_ab[t] = eye * H0A[t]
        taps_ab[6 + t] = eye * H1A[t]
        taps_de[t] = eye * G0S[t]
        taps_de[6 + t] = eye * G1S[t]
    dwq = np.zeros((4, 9, 128, 128), np.float32)
    wq = {0: conv5_w, 1: conv5_w, 2: conv7_w, 3: conv9_w}
    for sb in range(4):
        wloc = wq[sb][sl, 0]
        for t in range(9):
            dwq[sb, t] = np.diag(wloc[:, t // 3, t % 3])
    projlt = proj_w[:, sl].T.copy()  # (128, 256)
    tempv = np.repeat(np.asarray(temperature).reshape(8)[4 * g:4 * g + 4], 32).astype(np.float32)[:, None]
    return {
        "xk0": xb[:128].astype(bf), "xk1": xb[128:].astype(bf), "xq": xb[sl].astype(bf),
        "w1x1": w1.astype(bf), "dwk": dwk.astype(bf), "dwv": dwv.astype(bf),
        "taps_ab": taps_ab.astype(bf), "taps_de": taps_de.astype(bf),
        "dwq": dwq.astype(bf), "projlt": projlt.astype(bf), "tempv": tempv,
        "identb": np.eye(128, dtype=np.float32).astype(bf),
    }


def kernel(x, qkv_w, qkv_conv_w, conv5_w, conv7_w, conv9_w, proj_w, temperature, num_heads):
    x = np.asarray(x, np.float32)
    args = [np.asarray(a, np.float32) for a in
            (qkv_w, qkv_conv_w, conv5_w, conv7_w, conv9_w, proj_w)]
    temperature = np.asarray(temperature, np.float32)
    nc = build_core_kernel()
    in_maps = [_prep_core(x, *args, temperature, core // 2, core % 2) for core in range(8)]
    res = run_bass_kernel_spmd(nc, in_maps, core_ids=list(range(8)))
    out = np.zeros((4, 256, H, W), np.float32)
    for b in range(4):
        acc = res.results[2 * b]["y"].astype(np.float32) + res.results[2 * b + 1]["y"].astype(np.float32)
        out[b] = acc.reshape(256, H, W)
    return out


# revision 25
# speedup vs baseline: 1.0084x; 1.0084x over previous
"""MDTA Trainium2 kernel: 8 cores = 4 samples x 2 head-groups.

v1: unfolded qkv (1x1 + banded depthwise-3x3 fused in SBUF), norms fused
into producer stages, gram via DMA-transpose + SBUF-resident q^T/k^T,
bf16 inputs/outputs, per-iteration memsets eliminated, PSUM-evacuation
spread across Activation/DVE/Pool engines.
"""
import numpy as np
import ml_dtypes
import json as _json
import concourse.bass as bass

# Patch Bass.to_json_bytes: split multi-sem-waits onto same-engine NoOps
# (this walrus build rejects instructions with >1 sync wait).
_orig_tjb = bass.Bass.to_json_bytes
_wctr = [0]

def _split_waits(block):
    out = []
    for ins in block.get("instructions", []):
        si = ins.get("sync_info")
        waits = (si or {}).get("on_wait") or []
        if len(waits) > 1:
            si["on_wait"] = waits[-1:]
            for w in waits[:-1]:
                _wctr[0] += 1
                out.append({"debug": ins.get("debug", 0), "engine": ins["engine"],
                            "ins": [], "outs": [], "name": f"wsplit-{_wctr[0]}",
                            "opcode": "NoOp",
                            "sync_info": {"on_update": [], "on_wait": [w]}})
        out.append(ins)
    block["instructions"] = out
    for sub in block.get("blocks", []):
        _split_waits(sub)

def _patched_tjb(self):
    d = _json.loads(_orig_tjb(self))
    for fn in d.get("functions", []):
        for b in fn.get("blocks", []):
            _split_waits(b)
    return _json.dumps(d).encode()

if not getattr(bass.Bass, "_waitfix_done", False):
    bass.Bass.to_json_bytes = _patched_tjb
    bass.Bass._waitfix_done = True
import concourse.mybir as mybir
from concourse.tile import TileContext
from concourse.bass_utils import run_bass_kernel_spmd

BF = mybir.dt.bfloat16
F32 = mybir.dt.float32
H, W, C = 192, 192, 256
HW = H * W
S = 98  # subband size

DEC_LO = np.array([0.035226291882100656, -0.085441273882241486, -0.13501102001039084,
                   0.45987750211933132, 0.80689150931333875, 0.33267055295095688], dtype=np.float64)
DEC_HI = np.array([-0.33267055295095688, 0.80689150931333875, -0.45987750211933132,
                   -0.13501102001039084, 0.085441273882241486, 0.035226291882100656], dtype=np.float64)
H0A = DEC_LO[::-1].copy()
H1A = DEC_HI[::-1].copy()
G0S = DEC_LO.copy()  # REC_LO reversed = DEC_LO
G1S = np.array([0.035226291882100656, 0.085441273882241486, -0.13501102001039084,
                -0.45987750211933132, 0.80689150931333875, -0.33267055295095688], dtype=np.float64)[::-1].copy()


STAGE_MARKS = []


def _mark(nc, stage):
    STAGE_MARKS.append((int(nc.get_next_instruction_name().split("-")[1]), stage))


_rot = [0]


def _copy(nc, out, in_):
    # PSUM evacuation: GPSIMD/Pool cannot access PSUM, so rotate Act/DVE only.
    r = _rot[0] % 2
    _rot[0] += 1
    if r == 0:
        nc.scalar.copy(out, in_)
    else:
        nc.vector.tensor_copy(out, in_)


def build_core_kernel():
    nc = bass.Bass("TRN2")
    STAGE_MARKS.clear()
    _rot[0] = 0
    # inputs (per core)
    xk0 = nc.dram_tensor("xk0", [128, H, W], BF, kind="ExternalInput")
    xk1 = nc.dram_tensor("xk1", [128, H, W], BF, kind="ExternalInput")
    xq = nc.dram_tensor("xq", [128, H, W], BF, kind="ExternalInput")  # local 128 ch
    w1x1 = nc.dram_tensor("w1x1", [2, 128, 256], BF, kind="ExternalInput")  # [in-half, in, out(k|v)]
    dwk = nc.dram_tensor("dwk", [9, 128, 128], BF, kind="ExternalInput")  # diag dw taps, k half
    dwv = nc.dram_tensor("dwv", [9, 128, 128], BF, kind="ExternalInput")  # diag dw taps, v half
    taps_ab = nc.dram_tensor("taps_ab", [12, 128, 128], BF, kind="ExternalInput")
    taps_de = nc.dram_tensor("taps_de", [12, 128, 128], BF, kind="ExternalInput")
    dwq = nc.dram_tensor("dwq", [4, 9, 128, 128], BF, kind="ExternalInput")
    projlt = nc.dram_tensor("projlt", [128, 256], BF, kind="ExternalInput")
    tempv = nc.dram_tensor("tempv", [128, 1], F32, kind="ExternalInput")
    identb = nc.dram_tensor("identb", [128, 128], BF, kind="ExternalInput")
    y = nc.dram_tensor("y", [2, 128, HW], BF, kind="ExternalOutput")
    # DRAM scratch
    vd = nc.dram_tensor("vd", [128, HW], BF)
    qd = nc.dram_tensor("qd", [128, H, W], BF)
    loh = nc.dram_tensor("loh", [128, 2, H, S], BF)
    subb = nc.dram_tensor("subb", [128, 4, S, 100], BF)  # W-padded (cols 0,99 zeroed once)
    zq = nc.dram_tensor("zq", [128, 4, S, S], BF)

    with TileContext(nc) as tc:
        with tc.tile_pool(name="const", bufs=1) as cpool:
            # ---- constants
            t_ab = cpool.tile([128, 12, 128], BF)
            nc.sync.dma_start(out=t_ab[:, :, :], in_=taps_ab.rearrange("t p c -> p t c"))
            t_de = cpool.tile([128, 12, 128], BF)
            nc.sync.dma_start(out=t_de[:, :, :], in_=taps_de.rearrange("t p c -> p t c"))
            t_dw = cpool.tile([128, 36, 128], BF)
            nc.sync.dma_start(out=t_dw[:, :, :], in_=dwq.rearrange("s t p c -> p (s t) c"))
            t_w1 = cpool.tile([128, 2, 256], BF)
            nc.sync.dma_start(out=t_w1[:, :, :], in_=w1x1.rearrange("h p c -> p h c"))
            t_dwk = cpool.tile([128, 9, 128], BF)
            nc.sync.dma_start(out=t_dwk[:, :, :], in_=dwk.rearrange("t p c -> p t c"))
            t_dwv = cpool.tile([128, 9, 128], BF)
            nc.sync.dma_start(out=t_dwv[:, :, :], in_=dwv.rearrange("t p c -> p t c"))
            t_proj = cpool.tile([128, 256], BF)
            nc.sync.dma_start(out=t_proj[:, :], in_=projlt[:, :])
            t_id = cpool.tile([128, 128], BF)
            nc.sync.dma_start(out=t_id[:, :], in_=identb[:, :])
            t_temp = cpool.tile([128, 1], F32)
            nc.sync.dma_start(out=t_temp[:, :], in_=tempv[:, :])

            knp = cpool.tile([128, 16], F32)
            qnp = cpool.tile([128, 20], F32)
            nc.vector.memset(knp[:, :], 0)
            nc.vector.memset(qnp[:, :], 0)
            mt_ = cpool.tile([128, 256], BF)     # attention+proj matrix (gram phase -> y phase)
            kdT = cpool.tile([128, 288, 128], BF)  # transposed k, SBUF-resident

            _mark(nc, "kv")
            # ======== kv: 1x1 (C=256 -> k|v 128+128) + depthwise 3x3, fused per 12-row band
            BKV, NB = 12, 16
            with tc.tile_pool(name="kvw", bufs=2) as kp, \
                 tc.tile_pool(name="kvps", bufs=1, space="PSUM") as pp1:
                for b in range(NB):
                    r0 = BKV * b
                    xb0 = kp.tile([128, 14, 192], BF, tag="xb0")
                    xb1 = kp.tile([128, 14, 192], BF, tag="xb1")
                    v0, v1 = max(0, r0 - 1), min(H, r0 + BKV + 1)
                    if b == 0:
                        nc.vector.memset(xb0[:, 0, :], 0)
                        nc.vector.memset(xb1[:, 0, :], 0)
                    if b == NB - 1:
                        nc.vector.memset(xb0[:, 13, :], 0)
                        nc.vector.memset(xb1[:, 13, :], 0)
                    nc.sync.dma_start(out=xb0[:, v0 - (r0 - 1):v1 - (r0 - 1), :], in_=xk0[:, v0:v1, :])
                    nc.sync.dma_start(out=xb1[:, v0 - (r0 - 1):v1 - (r0 - 1), :], in_=xk1[:, v0:v1, :])
                    kvp = [kp.tile([128, 14, 194], BF, tag="kvp0", name="kvp0"),
                           kp.tile([128, 14, 194], BF, tag="kvp1", name="kvp1")]
                    for mt in range(2):  # zero W-pad columns (tiny, every band)
                        nc.vector.memset(kvp[mt][:, :, 0:1], 0)
                        nc.vector.memset(kvp[mt][:, :, 193:194], 0)
                    for mt in range(2):
                        for i in range(7):
                            ps = pp1.tile([128, 2, 192], F32, tag="ps1", bufs=2, name="ps1")
                            nc.tensor.matmul(ps[:, :, :], t_w1[:, 0, 128 * mt:128 * mt + 128],
                                             xb0[:, 2 * i:2 * i + 2, :], start=True, stop=False)
                            nc.tensor.matmul(ps[:, :, :], t_w1[:, 1, 128 * mt:128 * mt + 128],
                                             xb1[:, 2 * i:2 * i + 2, :], start=False, stop=True)
                            _copy(nc, kvp[mt][:, 2 * i:2 * i + 2, 1:193], ps[:, :, :])
                    for mt in range(2):
                        wt = t_dwk if mt == 0 else t_dwv
                        psd = [pp1.tile([128, 2, 192], F32, tag="psdw", bufs=6, name="psd")
                               for _ in range(6)]
                        for t9 in range(9):
                            u, v = divmod(t9, 3)
                            for j in range(6):
                                nc.tensor.matmul(psd[j][:, :, :], wt[:, t9, :],
                                                 kvp[mt][:, 2 * j + u:2 * j + u + 2, v:v + 192],
                                                 start=(t9 == 0), stop=(t9 == 8))
                        out = kp.tile([128, 12, 192], BF, tag=f"okv{mt}", name="okv")
                        for j in range(6):
                            _copy(nc, out[:, 2 * j:2 * j + 2, :], psd[j][:, :, :])
                        if mt == 0:
                            sqk = kp.tile([128, 12, 192], BF, tag="sqk")
                            nc.vector.scalar_tensor_tensor(sqk[:, :, :], out[:, :, :], 1.0,
                                                           out[:, :, :], mybir.AluOpType.mult,
                                                           mybir.AluOpType.mult,
                                                           accum_out=knp[:, b:b + 1])
                            nc.sync.dma_start_transpose(
                                out=kdT[:, 18 * b:18 * b + 18, :],
                                in_=out.rearrange("p r w -> p (r w)"))
                        else:
                            nc.sync.dma_start(out=vd[:, r0 * W:(r0 + BKV) * W],
                                              in_=out.rearrange("p r w -> p (r w)"))
                    if b == 0:  # zero subb (pad cols) via Act queue, overlapped with kv
                        zsrc = kp.tile([128, 1960], BF, tag="zsrc", bufs=1)
                        nc.vector.memset(zsrc[:, :], 0)
                        for sb in range(4):
                            for j in range(5):
                                nc.scalar.dma_start(
                                    out=subb[:, sb, :, :].rearrange("p r w -> p (r w)")[:, 1960 * j:1960 * j + 1960],
                                    in_=zsrc[:, :])

            # ======== wavelet query path (c-parts diag matmuls)
            with tc.tile_pool(name="ww", bufs=2) as pool, \
                 tc.tile_pool(name="wps", bufs=7, space="PSUM") as pp:
                _mark(nc, "stageA")
                # stage A: W-analysis (x -> lo/hi)
                RA = 5
                for it, r0 in enumerate(range(0, H, RA)):
                    rr = min(RA, H - r0)
                    xt = pool.tile([128, RA, 202], BF, tag="xa", bufs=4)
                    nc.vector.memset(xt[:, :rr, 0:4], 0)
                    nc.vector.memset(xt[:, :rr, 196:202], 0)
                    nc.sync.dma_start(out=xt[:, :rr, 4:196], in_=xq[:, r0:r0 + rr, :])
                    for f in range(2):
                        ps = pp.tile([128, RA, S], F32, tag="ps")
                        for t in range(6):
                            rhs = xt[:, :rr, t:t + 196].rearrange("p r (j two) -> p two r j", two=2)[:, 0]
                            nc.tensor.matmul(ps[:, :rr, :], t_ab[:, 6 * f + t, :], rhs,
                                             start=(t == 0), stop=(t == 5))
                        ot = pool.tile([128, RA, S], BF, tag="oA", bufs=3)
                        _copy(nc, ot[:, :rr, :], ps[:, :rr, :])
                        nc.sync.dma_start(out=loh[:, f, r0:r0 + rr, :], in_=ot[:, :rr, :])

                _mark(nc, "stageB")
                # stage B: H-analysis (lo/hi -> 4 subbands)
                RB = 5
                for m0 in range(0, S, RB):
                    mm = min(RB, S - m0)
                    lo_r0 = 2 * m0 - 4
                    nrows = 2 * mm + 5
                    bt = pool.tile([128, 2, 2 * RB + 5, S], BF, tag="xb", bufs=4)
                    v0 = max(0, lo_r0)
                    v1 = min(H, lo_r0 + nrows)
                    if v0 > lo_r0 or v1 < lo_r0 + nrows:
                        nc.vector.memset(bt[:, :, :, :], 0)
                    nc.sync.dma_start(out=bt[:, :, v0 - lo_r0:v1 - lo_r0, :],
                                      in_=loh[:, :, v0:v1, :])
                    for sb in range(4):
                        f_h, src = (sb % 2), (sb // 2)
                        ps = pp.tile([128, RB, S], F32, tag="ps")
                        for t in range(6):
                            rhs = bt[:, src, t:t + 2 * mm, :].rearrange("p (m two) j -> p two m j", two=2)[:, 0]
                            nc.tensor.matmul(ps[:, :mm, :], t_ab[:, 6 * f_h + t, :], rhs,
                                             start=(t == 0), stop=(t == 5))
                        ot = pool.tile([128, RB, S], BF, tag="oB", bufs=3)
                        _copy(nc, ot[:, :mm, :], ps[:, :mm, :])
                        nc.sync.dma_start(out=subb[:, sb, m0:m0 + mm, 1:99], in_=ot[:, :mm, :])

                _mark(nc, "stageC")
                # stage C: depthwise 3x3, all 4 subbands per row-chunk (one shared load)
                RC_ = 5
                for it, m0 in enumerate(range(0, S, RC_)):
                    mm = min(RC_, S - m0)
                    ct = pool.tile([128, 4, RC_ + 2, 100], BF, tag="xc", bufs=4)
                    v0 = max(0, m0 - 1)
                    v1 = min(S, m0 + mm + 1)
                    if v0 > m0 - 1 or v1 < m0 + mm + 1:
                        nc.vector.memset(ct[:, :, :, :], 0)
                    nc.sync.dma_start(out=ct[:, :, v0 - (m0 - 1):v1 - (m0 - 1), :],
                                      in_=subb[:, :, v0:v1, :])
                    for sb in range(4):
                        ps = pp.tile([128, RC_, S], F32, tag="ps")
                        for u in range(3):
                            for v in range(3):
                                t = 3 * u + v
                                nc.tensor.matmul(ps[:, :mm, :], t_dw[:, 9 * sb + t, :],
                                                 ct[:, sb, u:u + mm, v:v + 98],
                                                 start=(t == 0), stop=(t == 8))
                        ot = pool.tile([128, RC_, S], BF, tag="oC", bufs=3)
                        _copy(nc, ot[:, :mm, :], ps[:, :mm, :])
                        nc.sync.dma_start(out=zq[:, sb, m0:m0 + mm, :], in_=ot[:, :mm, :])

                _mark(nc, "stageD")
                # stages D+E fused: H-synthesis to an SBUF band, immediately W-synthesized
                # into q rows (no synth DRAM round-trip)
                RD = 5
                for a0 in range(0, 96, RD):
                    aa = min(RD, 96 - a0)
                    dt_ = pool.tile([128, 4, RD + 2, S], BF, tag="xd", bufs=4)
                    v1 = min(S, a0 + aa + 2)
                    nc.sync.dma_start(out=dt_[:, :, :v1 - a0, :],
                                      in_=zq[:, :, a0:v1, :])
                    sy = pool.tile([128, 2, 2, RD, S], BF, tag="sy", bufs=2)
                    for fo in range(2):
                        for pr in range(2):
                            ps = pp.tile([128, RD, S], F32, tag="ps")
                            for src in range(2):
                                for d in range(3):
                                    ti = 6 * src + (2 * d + 1 - pr)
                                    nc.tensor.matmul(ps[:, :aa, :], t_de[:, ti, :],
                                                     dt_[:, 2 * fo + src, d:d + aa, :],
                                                     start=(src == 0 and d == 0),
                                                     stop=(src == 1 and d == 2))
                            _copy(nc, sy[:, fo, pr, :aa, :], ps[:, :aa, :])
                    qsb = pool.tile([128, 2 * RD, W], BF, tag="qE")
                    for pr in range(2):
                        for pc in range(2):
                            ps = pp.tile([128, RD, 96], F32, tag="ps")
                            for src in range(2):
                                for d in range(3):
                                    ti = 6 * src + (2 * d + 1 - pc)
                                    nc.tensor.matmul(ps[:, :aa, :], t_de[:, ti, :],
                                                     sy[:, src, pr, :aa, d:d + 96],
                                                     start=(src == 0 and d == 0),
                                                     stop=(src == 1 and d == 2))
                            dst = qsb.rearrange("p (r two) w -> p two r w", two=2)[:, pr, :aa]
                            dst2 = dst.rearrange("p r (j two) -> p two r j", two=2)[:, pc]
                            _copy(nc, dst2, ps[:, :aa, :])
                    sqq = pool.tile([128, 2 * RD, W], BF, tag="sqq")
                    nc.vector.scalar_tensor_tensor(sqq[:, :2 * aa, :], qsb[:, :2 * aa, :], 1.0,
                                                   qsb[:, :2 * aa, :], mybir.AluOpType.mult,
                                                   mybir.AluOpType.mult,
                                                   accum_out=qnp[:, a0 // RD:a0 // RD + 1])
                    nc.sync.dma_start(out=qd[:, 2 * a0:2 * a0 + 2 * aa, :], in_=qsb[:, :2 * aa, :])

            _mark(nc, "gram")
            # ======== gram + attention block
            with tc.tile_pool(name="gw", bufs=1) as gw, \
                 tc.tile_pool(name="gps", bufs=1, space="PSUM") as gp:
                qdT = gw.tile([128, 288, 128], BF)
                g_ps = gp.tile([128, 128], F32, tag="g")
                qd_flat = qd.rearrange("p h w -> p (h w)")
                for c in range(8):
                    nc.sync.dma_start_transpose(out=qdT[:, 36 * c:36 * c + 36, :],
                                                in_=qd_flat[:, 4608 * c:4608 * c + 4608])
                    for i in range(36 * c, 36 * c + 36):
                        nc.tensor.matmul(g_ps[:, :], qdT[:, i, :], kdT[:, i, :],
                                         start=(i == 0), stop=(i == 287))

                _mark(nc, "attn")
                qn = gw.tile([128, 1], F32)
                kn = gw.tile([128, 1], F32)
                nc.vector.tensor_reduce(qn[:, :], qnp[:, :], axis=mybir.AxisListType.X,
                                        op=mybir.AluOpType.add)
                nc.vector.tensor_reduce(kn[:, :], knp[:, :], axis=mybir.AxisListType.X,
                                        op=mybir.AluOpType.add)
                nc.scalar.sqrt(qn[:, :], qn[:, :])
                nc.scalar.sqrt(kn[:, :], kn[:, :])
                nc.vector.tensor_scalar_max(qn[:, :], qn[:, :], 1e-12)
                nc.vector.tensor_scalar_max(kn[:, :], kn[:, :], 1e-12)
                rq = gw.tile([128, 1], F32)
                rk = gw.tile([128, 1], F32)
                nc.vector.reciprocal(rq[:, :], qn[:, :])
                nc.vector.reciprocal(rk[:, :], kn[:, :])
                nc.vector.tensor_mul(rq[:, :], rq[:, :], t_temp[:, :])

                gsb = gw.tile([128, 128], BF)
                nc.vector.tensor_scalar_mul(gsb[:, :], g_ps[:, :], rq[:, :])
                pt = gp.tile([128, 128], BF, tag="pt", bufs=2)
                nc.tensor.transpose(pt[:, :], gsb[:, :], t_id[:, :])
                gtb = gw.tile([128, 128], BF)
                nc.scalar.activation(gtb[:, :], pt[:, :], mybir.ActivationFunctionType.Copy,
                                     scale=rk[:, :])
                pt2 = gp.tile([128, 128], BF, tag="pt", bufs=2)
                nc.tensor.transpose(pt2[:, :], gtb[:, :], t_id[:, :])

                eb = gw.tile([128, 32], F32)
                for h in range(4):
                    nc.scalar.activation(eb[32 * h:32 * h + 32, :],
                                         pt2[32 * h:32 * h + 32, 32 * h:32 * h + 32],
                                         mybir.ActivationFunctionType.Exp)
                ssum = gw.tile([128, 1], F32)
                nc.vector.tensor_reduce(ssum[:, :], eb[:, :], axis=mybir.AxisListType.X,
                                        op=mybir.AluOpType.add)
                rs = gw.tile([128, 1], F32)
                nc.vector.reciprocal(rs[:, :], ssum[:, :])
                nc.vector.tensor_scalar_mul(eb[:, :], eb[:, :], rs[:, :])
                bd = gw.tile([128, 128], BF)
                nc.vector.memset(bd[:, :], 0)
                for h in range(4):
                    nc.vector.tensor_copy(bd[32 * h:32 * h + 32, 32 * h:32 * h + 32],
                                          eb[32 * h:32 * h + 32, :])

                mps = gp.tile([128, 256], F32, tag="mps")
                nc.tensor.matmul(mps[:, :], bd[:, :], t_proj[:, :], start=True, stop=True)
                nc.scalar.copy(mt_[:, :], mps[:, :])

            _mark(nc, "y")
            # ======== y = M @ v (2KB-per-partition DMA granularity)
            with tc.tile_pool(name="yw", bufs=2) as yp, \
                 tc.tile_pool(name="yps", bufs=8, space="PSUM") as yq:
                for i in range(18):
                    vt = yp.tile([128, 2048], BF, tag="vt", bufs=6)
                    nc.sync.dma_start(out=vt[:, :], in_=vd[:, 2048 * i:2048 * i + 2048])
                    yst = [yp.tile([128, 2048], BF, tag="yst0", name="yst0", bufs=3),
                           yp.tile([128, 2048], BF, tag="yst1", name="yst1", bufs=3)]
                    for j in range(4):
                        for mtile in range(2):
                            ps = yq.tile([128, 512], F32, tag="ps")
                            nc.tensor.matmul(ps[:, :], mt_[:, 128 * mtile:128 * mtile + 128],
                                             vt[:, 512 * j:512 * j + 512], start=True, stop=True)
                            _copy(nc, yst[mtile][:, 512 * j:512 * j + 512], ps[:, :])
                    for mtile in range(2):
                        nc.sync.dma_start(out=y[mtile, :, 2048 * i:2048 * i + 2048],
                                          in_=yst[mtile][:, :])
    return nc


def _prep_core(x, qkv_w, qkv_conv_w, conv5_w, conv7_w, conv9_w, proj_w, temperature, b, g):
    bf = ml_dtypes.bfloat16
    xb = np.asarray(x[b], np.float32)
    sl = slice(128 * g, 128 * g + 128)
    qkv_loc = np.concatenate([qkv_w[sl], qkv_w[256 + 128 * g:256 + 128 * g + 128]], 0)  # (256 out, 256 in)
    conv_loc = np.concatenate([qkv_conv_w[sl, 0], qkv_conv_w[256 + 128 * g:256 + 128 * g + 128, 0]], 0)  # (256,3,3)
    w1 = qkv_loc.T.reshape(2, 128, 256).copy()  # [in-half, in(128), out(256)]
    dwk = np.zeros((9, 128, 128), np.float32)
    dwv = np.zeros((9, 128, 128), np.float32)
    for t in range(9):
        u, v = divmod(t, 3)
        dwk[t] = np.diag(conv_loc[:128, u, v])
        dwv[t] = np.diag(conv_loc[128:, u, v])
    taps_ab = np.zeros((12, 128, 128), np.float32)
    taps_de = np.zeros((12, 128, 128), np.float32)
    eye = np.eye(128, dtype=np.float32)
    for t in range(6):
        taps_ab[t] = eye * H0A[t]
        taps_ab[6 + t] = eye * H1A[t]
        taps_de[t] = eye * G0S[t]
        taps_de[6 + t] = eye * G1S[t]
    dwq = np.zeros((4, 9, 128, 128), np.float32)
    wq = {0: conv5_w, 1: conv5_w, 2: conv7_w, 3: conv9_w}
    for sb in range(4):
        wloc = wq[sb][sl, 0]
        for t in range(9):
            dwq[sb, t] = np.diag(wloc[:, t // 3, t % 3])
    projlt = proj_w[:, sl].T.copy()  # (128, 256)
    tempv = np.repeat(np.asarray(temperature).reshape(8)[4 * g:4 * g + 4], 32).astype(np.float32)[:, None]
    return {
        "xk0": xb[:128].astype(bf), "xk1": xb[128:].astype(bf), "xq": xb[sl].astype(bf),
        "w1x1": w1.astype(bf), "dwk": dwk.astype(bf), "dwv": dwv.astype(bf),
        "taps_ab": taps_ab.astype(bf), "taps_de": taps_de.astype(bf),
        "dwq": dwq.astype(bf), "projlt": projlt.astype(bf), "tempv": tempv,
        "identb": np.eye(128, dtype=np.float32).astype(bf),
    }


def kernel(x, qkv_w, qkv_conv_w, conv5_w, conv7_w, conv9_w, proj_w, temperature, num_heads):
    x = np.asarray(x, np.float32)
    args = [np.asarray(a, np.float32) for a in
            (qkv_w, qkv_conv_w, conv5_w, conv7_w, conv9_w, proj_w)]
    temperature = np.asarray(temperature, np.float32)
    nc = build_core_kernel()
    in_maps = [_prep_core(x, *args, temperature, core // 2, core % 2) for core in range(8)]
    res = run_bass_kernel_spmd(nc, in_maps, core_ids=list(range(8)))
    out = np.zeros((4, 256, H, W), np.float32)
    for b in range(4):
        acc = res.results[2 * b]["y"].astype(np.float32) + res.results[2 * b + 1]["y"].astype(np.float32)
        out[b] = acc.reshape(256, H, W)
    return out
